# revision 8
# baseline (speedup 1.0000x reference)
"""Trainium2 Bass kernel for nn_Decoder_36206574305918 (vq_codebook).

Math (per batch b):
    Xf = X[b].reshape(D, N).T                      # [N, D]
    xc = Xf @ C.T                                  # [N, K]
    sl = scale * (|Xf|^2 + |C|^2 - 2 xc)           # [N, K]
    A  = softmax_k(sl)                             # [N, K]
    E  = A.T @ Xf - (sum_n A).T * C                # [K, D]

Sharding: data-parallel over B, one batch per NeuronCore (8 cores).

Device pipeline per core (all-bf16 matmul path, f32 logits):
  - SWDGE cast-DMA loads X f32->bf16 into SBUF in natural [d, n] layout
  - HWDGE xbar DMA-transpose produces X^T bf16 tiles [n, d]
  - PE mm1: xc[n,k] with X-tile stationary, C^T moving (PSUM f32)
  - x2 via fused square+accumulate (DVE tensor_tensor_reduce / ACT Square)
  - softmax on [128, 16*32] f32 slabs (DVE + ACT exp)
  - PE mm2: E += A_tile.T @ XT_tile accumulated over all n-tiles in PSUM,
    s = sum_n A via a ones(-1) column matmul
  - E_final = E - s*C on DVE, DMA out
"""

import os
import numpy as np
import ml_dtypes

B, D, HH, WW, K = 8, 512, 128, 128, 32
N = HH * WW            # 16384
P = 128                # partitions
NCHUNK = D // P        # 4 contraction chunks
SUP = 2048             # n columns per super-tile
NT = SUP // P          # 16 n-tiles per super
NSUP = N // SUP        # 8 super-tiles

_nc_cache = {}
last_results = None    # BassKernelResults of the most recent run (for test.py)


def _build_nc():
    import concourse.bass as bass
    import concourse.bacc as bacc
    import concourse.tile as tile
    from concourse import mybir

    f32 = mybir.dt.float32
    bf16 = mybir.dt.bfloat16
    Alu = mybir.AluOpType
    Act = mybir.ActivationFunctionType
    Axis = mybir.AxisListType

    nc = bacc.Bacc(None)
    x = nc.dram_tensor("x", [D, N], f32, kind="ExternalInput")
    ct = nc.dram_tensor("ct", [D, K], bf16, kind="ExternalInput")      # C^T, bf16
    crep = nc.dram_tensor("crep", [P, 2 * K], f32, kind="ExternalInput")  # [c2 | scale] replicated
    cf = nc.dram_tensor("cf", [K, D], f32, kind="ExternalInput")       # C, f32
    out = nc.dram_tensor("out", [K, D], f32, kind="ExternalOutput")

    with tile.TileContext(nc) as tc:
        with (
            tc.tile_pool(name="consts", bufs=1) as consts,
            tc.tile_pool(name="xn", bufs=2) as xnp,
            tc.tile_pool(name="xt", bufs=2) as xtp,
            tc.tile_pool(name="slab", bufs=2) as slab,
            tc.tile_pool(name="small", bufs=2) as small,
            tc.tile_pool(name="scratch", bufs=4) as scratch,
            tc.tile_pool(name="apool", bufs=2) as apool,
            tc.tile_pool(name="fin", bufs=1) as finp,
            tc.tile_pool(name="xcps", bufs=2, space="PSUM") as xcps,
            tc.tile_pool(name="eps", bufs=1, space="PSUM") as epsp,
        ):
            # --- constants ---
            ct_sb = consts.tile([P, NCHUNK, K], bf16)
            nc.sync.dma_start(out=ct_sb, in_=ct.rearrange("(c p) k -> p c k", p=P))
            crep_sb = consts.tile([P, 2 * K], f32)
            nc.sync.dma_start(out=crep_sb, in_=crep[:, :])
            cf_sb = consts.tile([K, D], f32)
            nc.sync.dma_start(out=cf_sb, in_=cf[:, :])
            negones = consts.tile([P, 1], bf16)
            nc.vector.memset(negones, -1.0)

            c2b = crep_sb[:, 0:K].unsqueeze(1).broadcast_to([P, NT, K])
            scb = crep_sb[:, K:2 * K].unsqueeze(1).broadcast_to([P, NT, K])

            e_ps = epsp.tile([K, D], f32)
            s_ps = epsp.tile([K, 1], f32)
            e_fin = finp.tile([K, D], f32)

            for s in range(NSUP):
                # --- load (f32 -> bf16 cast in DMA, single SWDGE copy) ---
                xn = xnp.tile([P, NCHUNK, SUP], bf16)
                nc.gpsimd.dma_start(
                    out=xn,
                    in_=x[:, s * SUP:(s + 1) * SUP].rearrange("(c p) n -> p c n", p=P),
                )
                # --- transpose (xbar) ---
                # out[p, t, c, j] holds X[d=c*128+j, n=s*SUP + p*NT + t]
                xt = xtp.tile([P, NT, NCHUNK, P], bf16)
                for c in range(NCHUNK):
                    nc.sync.dma_start(out=xt[:, :, c, :], in_=xn[:, c, :], transpose=True)

                # XT tile t holds n in [t*128, (t+1)*128), partition p = n - t*128
                # (verified on HW). mm1 lhsT uses the matching contiguous slice.

                # --- mm1: xc[p, t, k] = sum_d X[d, t*128+p] * Ct[d, k] ---
                xc = xcps.tile([P, NT, K], f32)
                for t in range(NT):
                    for c in range(NCHUNK):
                        nc.tensor.matmul(
                            xc[:, t, :],
                            lhsT=xn[:, c, t * P:(t + 1) * P],
                            rhs=ct_sb[:, c, :],
                            start=(c == 0),
                            stop=(c == NCHUNK - 1),
                        )

                # --- x2[q, t] = sum_d X[d, n(q,t)]^2 (from XT tiles) ---
                x2 = small.tile([P, NT], f32)
                for t in range(NT):
                    xt_t = xt[:, t, :, :].rearrange("p c j -> p (c j)")  # [128, 512]
                    sq = scratch.tile([P, D], bf16)
                    if t % 2 == 0:
                        nc.vector.scalar_tensor_tensor(
                            out=sq, in0=xt_t, scalar=1.0, in1=xt_t,
                            op0=Alu.mult, op1=Alu.mult, accum_out=x2[:, t:t + 1],
                        )
                    else:
                        nc.scalar.activation(
                            out=sq, in_=xt_t, func=Act.Square,
                            accum_out=x2[:, t:t + 1],
                        )

                # --- softmax slabs [128, NT*K] f32 ---
                # p = c2 - 2*xc ; q = p + x2 ; sl = q * scale
                psl = slab.tile([P, NT, K], f32)
                nc.vector.scalar_tensor_tensor(
                    out=psl, in0=xc, scalar=-2.0, in1=c2b,
                    op0=Alu.mult, op1=Alu.add,
                )
                qsl = slab.tile([P, NT, K], f32)
                nc.vector.tensor_add(qsl, psl, x2.unsqueeze(2).broadcast_to([P, NT, K]))
                sl = slab.tile([P, NT, K], f32)
                nc.vector.tensor_mul(sl, qsl, scb)
                mneg = small.tile([P, NT], f32)
                nc.vector.tensor_reduce(mneg, sl, axis=Axis.X, op=Alu.max, negate=True)
                slm = slab.tile([P, NT, K], f32)
                nc.vector.tensor_add(slm, sl, mneg.unsqueeze(2).broadcast_to([P, NT, K]))
                aun = slab.tile([P, NT, K], f32)
                nc.scalar.activation(out=aun, in_=slm, func=Act.Exp)
                z = small.tile([P, NT], f32)
                nc.vector.tensor_reduce(z, aun, axis=Axis.X, op=Alu.add)
                rz = small.tile([P, NT], f32)
                nc.vector.reciprocal(rz, z)
                a_sb = apool.tile([P, NT, K], bf16)
                nc.vector.tensor_mul(a_sb, aun, rz.unsqueeze(2).broadcast_to([P, NT, K]))

                # --- mm2: E += A_t.T @ XT_t ; s_neg += A_t.T @ (-1) ---
                for t in range(NT):
                    first = (s == 0 and t == 0)
                    last = (s == NSUP - 1 and t == NT - 1)
                    nc.tensor.matmul(
                        e_ps,
                        lhsT=a_sb[:, t, :],
                        rhs=xt[:, t, :, :].rearrange("p c j -> p (c j)"),
                        start=first, stop=last,
                    )
                    nc.tensor.matmul(
                        s_ps,
                        lhsT=a_sb[:, t, :],
                        rhs=negones,
                        start=first, stop=last,
                    )

            # --- final: E_fin = C * (-s) + E = E - s*C ---
            nc.vector.scalar_tensor_tensor(
                out=e_fin, in0=cf_sb, scalar=s_ps, in1=e_ps,
                op0=Alu.mult, op1=Alu.add,
            )
            nc.sync.dma_start(out=out[:, :], in_=e_fin)

    nc.finalize()
    return nc


def _get_nc():
    if "nc" not in _nc_cache:
        _nc_cache["nc"] = _build_nc()
    return _nc_cache["nc"]


def kernel(**inputs) -> np.ndarray:
    global last_results
    X = np.ascontiguousarray(np.asarray(inputs["X"], dtype=np.float32))
    C = np.ascontiguousarray(np.asarray(inputs["codewords"], dtype=np.float32))
    scale = np.asarray(inputs["scale"], dtype=np.float32)

    # host-side tiny precompute (O(K*D))
    c2 = (C.astype(np.float64) ** 2).sum(1).astype(np.float32)          # [K]
    crep = np.concatenate(
        [np.tile(c2[None, :], (P, 1)), np.tile(scale[None, :], (P, 1))], axis=1
    ).astype(np.float32)                                                # [128, 2K]
    ct = np.ascontiguousarray(C.T).astype(ml_dtypes.bfloat16)           # [D, K]

    in_maps = [
        {
            "x": np.ascontiguousarray(X[b].reshape(D, N)),
            "ct": ct,
            "crep": crep,
            "cf": C,
        }
        for b in range(B)
    ]

    from concourse.bass_utils import run_bass_kernel_spmd

    nc = _get_nc()
    res = run_bass_kernel_spmd(
        nc,
        in_maps,
        core_ids=list(range(B)),
        trace=bool(int(os.environ.get("KERNEL_TRACE", "0"))),
    )
    last_results = res
    return np.stack([r["out"] for r in res.results], axis=0)


if __name__ == "__main__":
    rng = np.random.default_rng(0)
    X = rng.standard_normal((B, D, HH, WW), dtype=np.float32)
    C = rng.uniform(-0.01, 0.01, (K, D)).astype(np.float32)
    s = rng.uniform(-1, 0, (K,)).astype(np.float32)
    E = kernel(X=X, codewords=C, scale=s)
    print("out", E.shape, E.dtype)
